# revision 1
# baseline (speedup 1.0000x reference)
"""MoE (63 routed experts, top-7, 1 shared expert) Trainium2 Bass kernel.

Strategy (expert parallelism, per sharding hint):
  - Host: router matmul + softmax + top-k (tiny: 0.7 GFLOP vs 220 GFLOP of
    expert FFNs), token gather per expert.
  - Device (8 NeuronCores, SPMD): each core runs 9 "units" of identical
    shape: 8 routed-expert slots (64 slots globally = 63 experts + 1
    overflow slot) and 1 shared-expert slot over a 1/8 token slice.
    Each unit: h = gelu(XeT^T @ W1 + b1); y = gate * (h @ W2), with
    full-rate matmuls (float32r or bf16), GELU fused into the PSUM
    eviction on the scalar engine, gating fused into the PSUM eviction on
    the vector engine.  Weights are host-pretiled into chunk-contiguous
    layout so every DMA is a flat [128 x bytes] block.
  - Host: scatter-add gated expert outputs (+ gate*b2), add shared out,
    bias and residual.

Experts are assigned to slots by descending load rank with static per-unit
token capacities (CAPS); both matmul layers' free dim is the capacity, so
PE cost tracks actual expert load.  Overload spills into the spare 64th
slot and, beyond that, to an exact host-side FFN for the few excess
tokens.  Gating and b2 are applied on the host during the scatter.
"""

import os

import numpy as np

B, S, HID = 2, 2048, 1280
E = 63
I = 1280
TOP_K = 7
NCORES = 8
UNITS = 9          # 8 expert slots + 1 shared-expert slot
C = 512            # token capacity per expert slot
CM = C // 128      # 4
KO = HID // 128    # 10 contraction chunks
T = B * S          # 4096
TSH = T // NCORES  # 512 shared-expert tokens per core

W1CW = 256          # w1 chunk width along I (2 lhsT column groups)
W2CW = 256          # w2 chunk width along H (2 lhsT column groups)
N_W1C = I // W1CW   # 5
N_W2C = HID // W2CW  # 5

# Per-unit-index token capacities. Experts are assigned to slots by load
# rank (rank r -> core r%8, unit r//8), so unit j only ever sees the j-th
# bucket of the descending load distribution; caps cover the bucket maxima
# of any near-uniform routing with margin. Uncovered overflow goes to the
# spare slot 63 and, beyond that, to an exact host fallback.
CAPS = [512, 500, 484, 472, 460, 448, 440, 420, C]   # unit 8 = shared

# "f32r": fp32 data, full-rate float32r matmuls (most accurate).
# "bf16": bf16 weights+activations, fp32 accumulate (halves DMA traffic).
# "fp16": like bf16 but 4x finer mantissa; all values here are well within
#         fp16 range, so this is strictly more accurate at the same speed.
WORK_DTYPE = os.environ.get("MOE_WDT", "fp16")

_cache = {}


def _build_nc(wdt):
    import concourse.mybir as mybir
    import concourse.tile as tile
    from concourse import bacc

    f32 = mybir.dt.float32
    GELU = mybir.ActivationFunctionType.Gelu
    if wdt == "f32r":
        mdt = mybir.dt.float32r
        ddt = f32    # dram dtype for weight/activation tensors
        bufs = dict(xu=2, h1=2, w1c=3, w2c=3, ou=2)
    else:
        mdt = mybir.dt.float16 if wdt == "fp16" else mybir.dt.bfloat16
        ddt = mdt
        bufs = dict(xu=3, h1=3, w1c=4, w2c=4, ou=2)

    nc = bacc.Bacc(None, target_bir_lowering=False)

    xg_d = nc.dram_tensor("xg", [UNITS, 128, KO, C], ddt, kind="ExternalInput")
    w1_d = nc.dram_tensor("w1", [UNITS, N_W1C, 128, KO, W1CW], ddt,
                          kind="ExternalInput")
    b1_d = nc.dram_tensor("b1", [UNITS, 128, KO], f32, kind="ExternalInput")
    w2_d = nc.dram_tensor("w2", [UNITS, N_W2C, 128, KO, W2CW], ddt,
                          kind="ExternalInput")
    # transposed output: out[u, p, hk, c] = y[token c, h = hk*128+p]
    out_d = nc.dram_tensor("out", [UNITS, 128, KO, C], f32, kind="ExternalOutput")

    def cast(ap):
        return ap.bitcast(mdt) if wdt == "f32r" else ap

    with tile.TileContext(nc) as tc:
        with tc.tile_pool(name="xg_p", bufs=bufs["xu"]) as xg_p, \
             tc.tile_pool(name="h1_p", bufs=bufs["h1"]) as h1_p, \
             tc.tile_pool(name="w1_p", bufs=bufs["w1c"]) as w1_p, \
             tc.tile_pool(name="w2_p", bufs=bufs["w2c"]) as w2_p, \
             tc.tile_pool(name="out_p", bufs=bufs["ou"]) as out_p, \
             tc.tile_pool(name="sm_p", bufs=3) as sm_p, \
             tc.tile_pool(name="ps1_p", bufs=3, space="PSUM") as ps1_p, \
             tc.tile_pool(name="ps2_p", bufs=4, space="PSUM") as ps2_p:

            for u in range(UNITS):
                CAP = CAPS[u]
                w1cs = {}
                # first w1 chunk ahead of everything else the unit needs
                w1cs[0] = w1_p.tile([128, KO, W1CW], mdt, tag="w1c", name="w1c")
                nc.sync.dma_start(w1cs[0][:], cast(w1_d[u, 0]))
                xu = xg_p.tile([128, KO, C], mdt, tag="xu")
                # split halves so the first matmuls can start sooner
                nc.sync.dma_start(xu[:, :KO // 2, :CAP],
                                  cast(xg_d[u, :, :KO // 2, :CAP]))
                nc.sync.dma_start(xu[:, KO // 2:, :CAP],
                                  cast(xg_d[u, :, KO // 2:, :CAP]))
                b1u = sm_p.tile([128, KO], f32, tag="b1u")
                nc.sync.dma_start(b1u[:], b1_d[u])

                h1 = h1_p.tile([128, KO, C], mdt, tag="h1")

                # ---- mm1: h1[i, c] = gelu(sum_h W1[h,i] * X^T[h,c] + b1[i])
                for ic in range(N_W1C):
                    if ic not in w1cs:
                        w1cs[ic] = w1_p.tile([128, KO, W1CW], mdt, tag="w1c", name="w1c")
                        nc.sync.dma_start(w1cs[ic][:], cast(w1_d[u, ic]))
                    w1c = w1cs[ic]
                    for s in range(W1CW // 128):
                        i_out = ic * (W1CW // 128) + s
                        ps = ps1_p.tile([128, C], f32, tag="ps1")
                        for ko in range(KO):
                            nc.tensor.matmul(
                                ps[:, :CAP],
                                w1c[:, ko, s * 128:(s + 1) * 128],
                                xu[:, ko, :CAP],
                                start=(ko == 0),
                                stop=(ko == KO - 1),
                            )
                        nc.scalar.activation(
                            h1[:, i_out, :CAP], ps[:, :CAP], GELU,
                            bias=b1u[:, i_out:i_out + 1])

                # ---- mm2 (transposed): yT[h, c] = sum_i W2[i, h] * h1[i, c]
                # gating and b2 are applied on the host during scatter.
                oy = out_p.tile([128, KO, C], f32, tag="oy")
                for hcc in range(N_W2C):
                    w2c = w2_p.tile([128, KO, W2CW], mdt, tag="w2c")
                    nc.sync.dma_start(w2c[:], cast(w2_d[u, hcc]))
                    for s2 in range(W2CW // 128):
                        hk = hcc * (W2CW // 128) + s2
                        ps2 = ps2_p.tile([128, C], f32, tag="ps2")
                        for ko in range(KO):
                            nc.tensor.matmul(
                                ps2[:, :CAP],
                                w2c[:, ko, s2 * 128:(s2 + 1) * 128],
                                h1[:, ko, :CAP],
                                start=(ko == 0),
                                stop=(ko == KO - 1),
                            )
                        nc.vector.tensor_copy(oy[:, hk, :CAP], ps2[:, :CAP])
                        # drain finished output rows early so the final DMA
                        # (and the kernel tail) stays small
                        if hk % 2 == 1:
                            nc.sync.dma_start(
                                out_d[u, :, hk - 1:hk + 1, :CAP],
                                oy[:, hk - 1:hk + 1, :CAP])

    nc.compile()
    return nc


def _get_nc(wdt):
    if wdt not in _cache:
        _cache[wdt] = _build_nc(wdt)
    return _cache[wdt]


def _np_wdt(wdt):
    if wdt == "bf16":
        import ml_dtypes
        return np.dtype(ml_dtypes.bfloat16)
    if wdt == "fp16":
        return np.dtype(np.float16)
    return np.dtype(np.float32)


def _gelu_np(v):
    from scipy.special import erf
    v = v.astype(np.float32)
    return (0.5 * v * (1.0 + erf(v / np.sqrt(2.0)))).astype(np.float32)


def _tile_w1(w):
    # [H, I] -> [N_W1C, 128, KO, W1CW] with w1t[ic, p, ko, j] = w[ko*128+p, ic*W1CW+j]
    return w.reshape(KO, 128, N_W1C, W1CW).transpose(2, 1, 0, 3)


def _tile_w2(w):
    # [I, H] -> [N_W2C, 128, KO, W2CW]
    return w.reshape(KO, 128, N_W2C, W2CW).transpose(2, 1, 0, 3)


def _ensure_axon_hooks_stub():
    """bass_utils' axon trace path imports antenv.axon_hooks, which this
    image lacks; provide a no-op stub so a BASS_TRACE-enabled environment
    degrades gracefully instead of crashing."""
    import sys
    import types
    try:
        import antenv.axon_hooks  # noqa: F401
        return
    except ImportError:
        pass
    try:
        import antenv
    except ImportError:
        return
    mod = types.ModuleType("antenv.axon_hooks")
    holder = [None]
    mod.set_axon_ntff_profile_hook = lambda h: holder.__setitem__(0, h)
    mod.get_axon_ntff_profile_hook = lambda: holder[0]
    sys.modules["antenv.axon_hooks"] = mod
    antenv.axon_hooks = mod


def kernel(x, w1_shared, b1_shared, w2_shared, b2_shared,
           router_w, router_b, w1, b1, w2, b2):
    _ensure_axon_hooks_stub()
    from concourse.bass_utils import run_bass_kernel_spmd

    wdt = WORK_DTYPE
    ndt = _np_wdt(wdt)

    x = np.asarray(x, np.float32)
    w1 = np.asarray(w1, np.float32)
    b1 = np.asarray(b1, np.float32)
    w2 = np.asarray(w2, np.float32)
    b2 = np.asarray(b2, np.float32)
    w1_shared = np.asarray(w1_shared, np.float32)
    b1_shared = np.asarray(b1_shared, np.float32)
    w2_shared = np.asarray(w2_shared, np.float32)
    b2_shared = np.asarray(b2_shared, np.float32)
    router_w = np.asarray(router_w, np.float32)
    router_b = np.asarray(router_b, np.float32)

    xf = x.reshape(T, HID)

    # ---------------- host routing ----------------
    logits = xf @ router_w + router_b
    m = logits.max(-1, keepdims=True)
    ex = np.exp(logits - m, dtype=np.float32)
    affin = ex / ex.sum(-1, keepdims=True, dtype=np.float32)
    order = np.argsort(-affin, axis=-1, kind="stable")[:, :TOP_K]   # [T, K]
    vals = np.take_along_axis(affin, order, axis=-1)                # [T, K]

    # group (token, gate) pairs by expert
    flat_e = order.ravel()
    flat_t = np.repeat(np.arange(T), TOP_K)
    flat_g = vals.ravel()
    sort = np.argsort(flat_e, kind="stable")
    se, st, sg = flat_e[sort], flat_t[sort], flat_g[sort]
    starts = np.searchsorted(se, np.arange(E + 1))
    tok_by_e = [st[starts[e]:starts[e + 1]] for e in range(E)]
    gate_by_e = [sg[starts[e]:starts[e + 1]] for e in range(E)]

    # slot table: 64 expert slots; slot s = core*8 + unit.  Experts are
    # assigned by descending load rank: rank r -> core r%8, unit r//8, so
    # every core gets one expert from each load bucket and unit j's static
    # capacity CAPS[j] covers its bucket maximum.
    NSLOT = NCORES * 8
    slot_expert = [-1] * NSLOT
    slot_tok = [np.empty(0, np.int64)] * NSLOT
    slot_gate = [np.empty(0, np.float32)] * NSLOT
    ranked = sorted(range(E), key=lambda e: -len(tok_by_e[e]))
    overflow = []   # (expert, tokens, gates) beyond the primary slot cap
    for r, e in enumerate(ranked):
        s = (r % NCORES) * 8 + (r // NCORES)
        cap = CAPS[r // NCORES]
        slot_expert[s] = e
        slot_tok[s] = tok_by_e[e][:cap]
        slot_gate[s] = gate_by_e[e][:cap]
        if len(tok_by_e[e]) > cap:
            overflow.append((e, tok_by_e[e][cap:], gate_by_e[e][cap:]))
    # worst overflow spills into the spare slot 63 (unit 7, cap CAPS[7]);
    # anything further goes to an exact host fallback (rare).
    host_fallback = []
    if overflow:
        overflow.sort(key=lambda t: -len(t[1]))
        e0, t0, g0 = overflow[0]
        cap63 = CAPS[7]
        slot_expert[63] = e0
        slot_tok[63] = t0[:cap63]
        slot_gate[63] = g0[:cap63]
        if len(t0) > cap63:
            host_fallback.append((e0, t0[cap63:], g0[cap63:]))
        for e, t, g in overflow[1:]:
            host_fallback.append((e, t, g))

    # ---------------- build per-core device inputs ----------------
    # x transposed + partition-tiled: xT_t[ko, p, t] = x[t, ko*128+p]
    xT_t = np.ascontiguousarray(xf.T).astype(ndt).reshape(KO, 128, T)

    w1t_sh = _tile_w1(w1_shared[0]).astype(ndt)
    w2t_sh = _tile_w2(w2_shared[0]).astype(ndt)
    b1t_sh = b1_shared[0].reshape(KO, 128).T

    in_maps = []
    for c in range(NCORES):
        xg = np.zeros((UNITS, 128, KO, C), ndt)
        w1u = np.zeros((UNITS, N_W1C, 128, KO, W1CW), ndt)
        b1u = np.zeros((UNITS, 128, KO), np.float32)
        w2u = np.zeros((UNITS, N_W2C, 128, KO, W2CW), ndt)
        for u in range(8):
            s = c * 8 + u
            e = slot_expert[s]
            if e < 0 or len(slot_tok[s]) == 0:
                continue
            n = len(slot_tok[s])
            idx = np.zeros(C, np.int64)
            idx[:n] = slot_tok[s]
            xg[u] = xT_t[:, :, idx].swapaxes(0, 1)
            w1u[u] = _tile_w1(w1[e]).astype(ndt)
            b1u[u] = b1[e].reshape(KO, 128).T
            w2u[u] = _tile_w2(w2[e]).astype(ndt)
        # shared-expert unit
        xg[8] = xT_t[:, :, c * TSH:(c + 1) * TSH].swapaxes(0, 1)
        w1u[8] = w1t_sh
        b1u[8] = b1t_sh
        w2u[8] = w2t_sh
        in_maps.append({"xg": xg, "w1": w1u, "b1": b1u, "w2": w2u})

    # ---------------- run on 8 cores ----------------
    nc = _get_nc(wdt)
    res = run_bass_kernel_spmd(nc, in_maps, core_ids=list(range(NCORES)))
    outs = [r["out"] for r in res.results]   # [UNITS, 128, CM, HID] each

    # ---------------- host unshard / scatter ----------------
    # device output is transposed: outs[c][u][p, hk, c'] = y[c', hk*128+p]
    def untile_y(o, n):
        return o.transpose(1, 0, 2).reshape(HID, C)[:, :n].T

    acc = np.zeros((T, HID), np.float32)     # shared + routed
    # shared expert (unit 8 on each core), gate 1, + b2_shared
    for c in range(NCORES):
        ys = untile_y(outs[c][8], TSH)
        acc[c * TSH:(c + 1) * TSH] = ys + b2_shared[0]
    # routed experts: gate * (y + b2), scattered by token
    for s in range(NCORES * 8):
        e = slot_expert[s]
        n = len(slot_tok[s])
        if e < 0 or n == 0:
            continue
        ye = untile_y(outs[s // 8][s % 8], n)
        # token indices are unique within one slot, so fancy += is safe
        acc[slot_tok[s]] += slot_gate[s][:, None] * (ye + b2[e][None, :])
    # exact host fallback for overflow beyond device capacity
    for e, toks, gs in host_fallback:
        h = _gelu_np(xf[toks] @ w1[e] + b1[e])
        acc[toks] += gs[:, None] * (h @ w2[e] + b2[e])

    return (acc + xf).reshape(B, S, HID).astype(np.float32)



# revision 2
# speedup vs baseline: 1.3667x; 1.3667x over previous
"""MoE (63 routed experts, top-7, 1 shared expert) Trainium2 Bass kernel.

Strategy (expert parallelism, per sharding hint):
  - Host: router matmul + softmax + top-k (tiny: 0.7 GFLOP vs 220 GFLOP of
    expert FFNs), token gather per expert.
  - Device (8 NeuronCores, SPMD): each core runs 9 "units": 8 routed-expert
    slots (64 slots globally = 63 experts + 1 overflow slot) and 1
    shared-expert slot over a 1/8 token slice.
    Routed units run in fp8e4m3 with DoubleRow matmuls (2 fp8 weights per PE
    cell, K=256 per instruction): h = gelu((XeT^T @ (256*W1))/256 + b1);
    y*256 = h @ (256*W2).  Weights are pre-scaled by 256 (a power of two, so
    exact) to keep them in e4m3's normal range; the 1/256 is folded into the
    GELU's input scale on layer 1 and into the host-side gate scaling on
    layer 2.  The shared-expert unit (gate 1.0, so it dominates the error
    budget) runs in plain fp16.
  - Host: scatter-add gated expert outputs (+ gate*b2), add shared out,
    bias and residual.

Experts are assigned to slots by descending load rank with static per-unit
token capacities (CAPS); both matmul layers' free dim is the capacity, so
PE cost tracks actual expert load.  Overload spills into the spare 64th
slot and, beyond that, to an exact host-side FFN for the few excess
tokens.  Gating and b2 are applied on the host during the scatter.
"""

import os

import numpy as np

B, S, HID = 2, 2048, 1280
E = 63
I = 1280
TOP_K = 7
NCORES = 8
UNITS = 9          # 8 routed-expert slots + 1 shared-expert slot
RUNITS = 8         # routed units per core
C = 512            # token capacity per expert slot
KO = HID // 128    # 10 contraction chunks of 128
KP = KO // 2       # 5 DoubleRow contraction pairs (K=256 each)
T = B * S          # 4096
TSH = T // NCORES  # 512 shared-expert tokens per core

W1CW = 256          # w1 chunk width along I (2 lhsT column groups)
W2CW = 256          # w2 chunk width along H (2 lhsT column groups)
N_W1C = I // W1CW   # 5
N_W2C = HID // W2CW  # 5

WSCALE = 256.0      # power-of-two pre-scale for fp8 routed weights

# Per-unit-index token capacities. Experts are assigned to slots by load
# rank (rank r -> core r%8, unit r//8), so unit j only ever sees the j-th
# bucket of the descending load distribution; caps cover the bucket maxima
# of any near-uniform routing with margin. Uncovered overflow goes to the
# spare slot 63 and, beyond that, to an exact host fallback.
CAPS = [512, 500, 484, 472, 460, 448, 440, 420, C]   # unit 8 = shared

_cache = {}


def _build_nc():
    import concourse.mybir as mybir
    import concourse.tile as tile
    from concourse import bacc

    f32 = mybir.dt.float32
    f16 = mybir.dt.float16
    f8 = mybir.dt.float8e4
    GELU = mybir.ActivationFunctionType.Gelu
    DR = mybir.MatmulPerfMode.DoubleRow

    nc = bacc.Bacc(None, target_bir_lowering=False)

    # routed (fp8) inputs
    xg_d = nc.dram_tensor("xg", [RUNITS, 128, KO, C], f8, kind="ExternalInput")
    w1_d = nc.dram_tensor("w1", [RUNITS, N_W1C, 128, KO, W1CW], f8,
                          kind="ExternalInput")
    w2_d = nc.dram_tensor("w2", [RUNITS, N_W2C, 128, KO, W2CW], f8,
                          kind="ExternalInput")
    # shared (fp16) inputs
    xs_d = nc.dram_tensor("xs", [128, KO, TSH], f16, kind="ExternalInput")
    w1s_d = nc.dram_tensor("w1s", [N_W1C, 128, KO, W1CW], f16,
                           kind="ExternalInput")
    w2s_d = nc.dram_tensor("w2s", [N_W2C, 128, KO, W2CW], f16,
                           kind="ExternalInput")
    b1_d = nc.dram_tensor("b1", [UNITS, 128, KO], f32, kind="ExternalInput")
    # transposed output: out[u, p, hk, c] = yscaled[token c, h = hk*128+p]
    # routed units hold 256*y (host folds 1/256 into gates); shared holds y.
    out_d = nc.dram_tensor("out", [UNITS, 128, KO, C], f16, kind="ExternalOutput")

    with tile.TileContext(nc) as tc:
        with tc.tile_pool(name="xg_p", bufs=3) as xg_p, \
             tc.tile_pool(name="h1_p", bufs=3) as h1_p, \
             tc.tile_pool(name="w1_p", bufs=4) as w1_p, \
             tc.tile_pool(name="w2_p", bufs=4) as w2_p, \
             tc.tile_pool(name="out_p", bufs=2) as out_p, \
             tc.tile_pool(name="sm_p", bufs=3) as sm_p, \
             tc.tile_pool(name="ps1_p", bufs=3, space="PSUM") as ps1_p, \
             tc.tile_pool(name="ps2_p", bufs=4, space="PSUM") as ps2_p:

            for u in range(UNITS):
                CAP = CAPS[u]
                shared = (u == 8)
                mdt = f16 if shared else f8

                w1cs = {}
                # first w1 chunk ahead of everything else the unit needs
                w1cs[0] = w1_p.tile([128, KO, W1CW], mdt, tag="w1c", name="w1c")
                nc.sync.dma_start(w1cs[0][:], w1s_d[0] if shared else w1_d[u, 0])
                xu = xg_p.tile([128, KO, C], mdt, tag="xu")
                # split halves so the first matmuls can start sooner
                if shared:
                    nc.sync.dma_start(xu[:, :KO // 2, :CAP],
                                      xs_d[:, :KO // 2, :CAP])
                    nc.sync.dma_start(xu[:, KO // 2:, :CAP],
                                      xs_d[:, KO // 2:, :CAP])
                else:
                    nc.sync.dma_start(xu[:, :KO // 2, :CAP],
                                      xg_d[u, :, :KO // 2, :CAP])
                    nc.sync.dma_start(xu[:, KO // 2:, :CAP],
                                      xg_d[u, :, KO // 2:, :CAP])
                b1u = sm_p.tile([128, KO], f32, tag="b1u")
                nc.sync.dma_start(b1u[:], b1_d[u])

                h1 = h1_p.tile([128, KO, C], mdt, tag="h1")

                # ---- mm1: h1[i, c] = gelu(sum_h W1[h,i] * X^T[h,c] + b1[i])
                for ic in range(N_W1C):
                    if ic not in w1cs:
                        w1cs[ic] = w1_p.tile([128, KO, W1CW], mdt, tag="w1c",
                                             name="w1c")
                        nc.sync.dma_start(w1cs[ic][:],
                                          w1s_d[ic] if shared else w1_d[u, ic])
                    w1c = w1cs[ic]
                    for s in range(W1CW // 128):
                        i_out = ic * (W1CW // 128) + s
                        ps = ps1_p.tile([128, C], f32, tag="ps1")
                        if shared:
                            for ko in range(KO):
                                nc.tensor.matmul(
                                    ps[:, :CAP],
                                    w1c[:, ko, s * 128:(s + 1) * 128],
                                    xu[:, ko, :CAP],
                                    start=(ko == 0),
                                    stop=(ko == KO - 1),
                                )
                        else:
                            for j in range(KP):
                                nc.tensor.matmul(
                                    ps[:, :CAP],
                                    w1c[:, 2 * j:2 * j + 2, s * 128:(s + 1) * 128],
                                    xu[:, 2 * j:2 * j + 2, :CAP],
                                    start=(j == 0),
                                    stop=(j == KP - 1),
                                    perf_mode=DR,
                                )
                        nc.scalar.activation(
                            h1[:, i_out, :CAP], ps[:, :CAP], GELU,
                            bias=b1u[:, i_out:i_out + 1],
                            scale=1.0 if shared else 1.0 / WSCALE)

                # ---- mm2 (transposed): yT[h, c] = sum_i W2[i, h] * h1[i, c]
                # gating and b2 are applied on the host during scatter.
                oy = out_p.tile([128, KO, C], f16, tag="oy")
                for hcc in range(N_W2C):
                    w2c = w2_p.tile([128, KO, W2CW], mdt, tag="w2c")
                    nc.sync.dma_start(w2c[:], w2s_d[hcc] if shared else w2_d[u, hcc])
                    for s2 in range(W2CW // 128):
                        hk = hcc * (W2CW // 128) + s2
                        ps2 = ps2_p.tile([128, C], f32, tag="ps2")
                        if shared:
                            for ko in range(KO):
                                nc.tensor.matmul(
                                    ps2[:, :CAP],
                                    w2c[:, ko, s2 * 128:(s2 + 1) * 128],
                                    h1[:, ko, :CAP],
                                    start=(ko == 0),
                                    stop=(ko == KO - 1),
                                )
                        else:
                            for j in range(KP):
                                nc.tensor.matmul(
                                    ps2[:, :CAP],
                                    w2c[:, 2 * j:2 * j + 2, s2 * 128:(s2 + 1) * 128],
                                    h1[:, 2 * j:2 * j + 2, :CAP],
                                    start=(j == 0),
                                    stop=(j == KP - 1),
                                    perf_mode=DR,
                                )
                        nc.vector.tensor_copy(oy[:, hk, :CAP], ps2[:, :CAP])
                        # drain finished output rows early so the final DMA
                        # (and the kernel tail) stays small
                        if hk % 2 == 1:
                            nc.sync.dma_start(
                                out_d[u, :, hk - 1:hk + 1, :CAP],
                                oy[:, hk - 1:hk + 1, :CAP])

    nc.compile()
    return nc


def _get_nc():
    if "nc" not in _cache:
        _cache["nc"] = _build_nc()
    return _cache["nc"]


def _f8():
    import ml_dtypes
    return np.dtype(ml_dtypes.float8_e4m3)


def _gelu_np(v):
    from scipy.special import erf
    v = v.astype(np.float32)
    return (0.5 * v * (1.0 + erf(v / np.sqrt(2.0)))).astype(np.float32)


def _tile_w1(w):
    # [H, I] -> [N_W1C, 128, KO, W1CW] with w1t[ic, p, ko, j] = w[ko*128+p, ic*W1CW+j]
    return w.reshape(KO, 128, N_W1C, W1CW).transpose(2, 1, 0, 3)


def _tile_w2(w):
    # [I, H] -> [N_W2C, 128, KO, W2CW]
    return w.reshape(KO, 128, N_W2C, W2CW).transpose(2, 1, 0, 3)


def _ensure_axon_hooks_stub():
    """bass_utils' axon trace path imports antenv.axon_hooks, which this
    image lacks; provide a no-op stub so a BASS_TRACE-enabled environment
    degrades gracefully instead of crashing."""
    import sys
    import types
    try:
        import antenv.axon_hooks  # noqa: F401
        return
    except ImportError:
        pass
    try:
        import antenv
    except ImportError:
        return
    mod = types.ModuleType("antenv.axon_hooks")
    holder = [None]
    mod.set_axon_ntff_profile_hook = lambda h: holder.__setitem__(0, h)
    mod.get_axon_ntff_profile_hook = lambda: holder[0]
    sys.modules["antenv.axon_hooks"] = mod
    antenv.axon_hooks = mod


def kernel(x, w1_shared, b1_shared, w2_shared, b2_shared,
           router_w, router_b, w1, b1, w2, b2):
    _ensure_axon_hooks_stub()
    from concourse.bass_utils import run_bass_kernel_spmd

    f8 = _f8()

    x = np.asarray(x, np.float32)
    w1 = np.asarray(w1, np.float32)
    b1 = np.asarray(b1, np.float32)
    w2 = np.asarray(w2, np.float32)
    b2 = np.asarray(b2, np.float32)
    w1_shared = np.asarray(w1_shared, np.float32)
    b1_shared = np.asarray(b1_shared, np.float32)
    w2_shared = np.asarray(w2_shared, np.float32)
    b2_shared = np.asarray(b2_shared, np.float32)
    router_w = np.asarray(router_w, np.float32)
    router_b = np.asarray(router_b, np.float32)

    xf = x.reshape(T, HID)

    # ---------------- host routing ----------------
    logits = xf @ router_w + router_b
    m = logits.max(-1, keepdims=True)
    ex = np.exp(logits - m, dtype=np.float32)
    affin = ex / ex.sum(-1, keepdims=True, dtype=np.float32)
    order = np.argsort(-affin, axis=-1, kind="stable")[:, :TOP_K]   # [T, K]
    vals = np.take_along_axis(affin, order, axis=-1)                # [T, K]

    # group (token, gate) pairs by expert
    flat_e = order.ravel()
    flat_t = np.repeat(np.arange(T), TOP_K)
    flat_g = vals.ravel()
    sort = np.argsort(flat_e, kind="stable")
    se, st, sg = flat_e[sort], flat_t[sort], flat_g[sort]
    starts = np.searchsorted(se, np.arange(E + 1))
    tok_by_e = [st[starts[e]:starts[e + 1]] for e in range(E)]
    gate_by_e = [sg[starts[e]:starts[e + 1]] for e in range(E)]

    # slot table: 64 expert slots; slot s = core*8 + unit.  Experts are
    # assigned by descending load rank: rank r -> core r%8, unit r//8, so
    # every core gets one expert from each load bucket and unit j's static
    # capacity CAPS[j] covers its bucket maximum.
    NSLOT = NCORES * 8
    slot_expert = [-1] * NSLOT
    slot_tok = [np.empty(0, np.int64)] * NSLOT
    slot_gate = [np.empty(0, np.float32)] * NSLOT
    ranked = sorted(range(E), key=lambda e: -len(tok_by_e[e]))
    overflow = []   # (expert, tokens, gates) beyond the primary slot cap
    for r, e in enumerate(ranked):
        s = (r % NCORES) * 8 + (r // NCORES)
        cap = CAPS[r // NCORES]
        slot_expert[s] = e
        slot_tok[s] = tok_by_e[e][:cap]
        slot_gate[s] = gate_by_e[e][:cap]
        if len(tok_by_e[e]) > cap:
            overflow.append((e, tok_by_e[e][cap:], gate_by_e[e][cap:]))
    # worst overflow spills into the spare slot 63 (unit 7, cap CAPS[7]);
    # anything further goes to an exact host fallback (rare).
    host_fallback = []
    if overflow:
        overflow.sort(key=lambda t: -len(t[1]))
        e0, t0, g0 = overflow[0]
        cap63 = CAPS[7]
        slot_expert[63] = e0
        slot_tok[63] = t0[:cap63]
        slot_gate[63] = g0[:cap63]
        if len(t0) > cap63:
            host_fallback.append((e0, t0[cap63:], g0[cap63:]))
        for e, t, g in overflow[1:]:
            host_fallback.append((e, t, g))

    # ---------------- build per-core device inputs ----------------
    # x transposed + partition-tiled: xT_t[ko, p, t] = x[t, ko*128+p]
    xT = np.ascontiguousarray(xf.T)
    xT_t8 = xT.astype(f8).reshape(KO, 128, T)
    xT_t16 = xT.astype(np.float16).reshape(KO, 128, T)

    w1t_sh = _tile_w1(w1_shared[0]).astype(np.float16)
    w2t_sh = _tile_w2(w2_shared[0]).astype(np.float16)
    b1t_sh = b1_shared[0].reshape(KO, 128).T

    in_maps = []
    for c in range(NCORES):
        xg = np.zeros((RUNITS, 128, KO, C), f8)
        w1u = np.zeros((RUNITS, N_W1C, 128, KO, W1CW), f8)
        b1u = np.zeros((UNITS, 128, KO), np.float32)
        w2u = np.zeros((RUNITS, N_W2C, 128, KO, W2CW), f8)
        for u in range(RUNITS):
            s = c * 8 + u
            e = slot_expert[s]
            if e < 0 or len(slot_tok[s]) == 0:
                continue
            n = len(slot_tok[s])
            idx = np.zeros(C, np.int64)
            idx[:n] = slot_tok[s]
            xg[u] = xT_t8[:, :, idx].swapaxes(0, 1)
            w1u[u] = _tile_w1(w1[e] * WSCALE).astype(f8)
            b1u[u] = b1[e].reshape(KO, 128).T
            w2u[u] = _tile_w2(w2[e] * WSCALE).astype(f8)
        # shared-expert unit
        xs = xT_t16[:, :, c * TSH:(c + 1) * TSH].swapaxes(0, 1)
        b1u[8] = b1t_sh
        in_maps.append({"xg": xg, "w1": w1u, "b1": b1u, "w2": w2u,
                        "xs": np.ascontiguousarray(xs),
                        "w1s": w1t_sh, "w2s": w2t_sh})

    # ---------------- run on 8 cores ----------------
    nc = _get_nc()
    res = run_bass_kernel_spmd(nc, in_maps, core_ids=list(range(NCORES)))
    outs = [r["out"] for r in res.results]   # [UNITS, 128, KO, C] each

    # ---------------- host unshard / scatter ----------------
    # device output is transposed: outs[c][u][p, hk, c'] = ysc[c', hk*128+p]
    def untile_y(o, n):
        return o.transpose(1, 0, 2).reshape(HID, C)[:, :n].T.astype(np.float32)

    acc = np.zeros((T, HID), np.float32)     # shared + routed
    # shared expert (unit 8 on each core), gate 1, + b2_shared
    for c in range(NCORES):
        ys = untile_y(outs[c][8], TSH)
        acc[c * TSH:(c + 1) * TSH] = ys + b2_shared[0]
    # routed experts: gate * (y + b2), scattered by token; device holds
    # 256*y so fold the 1/256 into the gate.
    inv = np.float32(1.0 / WSCALE)
    for s in range(NCORES * 8):
        e = slot_expert[s]
        n = len(slot_tok[s])
        if e < 0 or n == 0:
            continue
        ye = untile_y(outs[s // 8][s % 8], n)
        # token indices are unique within one slot, so fancy += is safe
        acc[slot_tok[s]] += (slot_gate[s] * inv)[:, None] * ye \
            + slot_gate[s][:, None] * b2[e][None, :]
    # exact host fallback for overflow beyond device capacity
    for e, toks, gs in host_fallback:
        h = _gelu_np(xf[toks] @ w1[e] + b1[e])
        acc[toks] += gs[:, None] * (h @ w2[e] + b2[e])

    return (acc + xf).reshape(B, S, HID).astype(np.float32)


# revision 3
# speedup vs baseline: 1.6674x; 1.2200x over previous
"""MoE (63 routed experts, top-7, 1 shared expert) Trainium2 Bass kernel.

Strategy (expert parallelism, per sharding hint):
  - Host: router matmul + softmax + top-k (tiny: 0.7 GFLOP vs 220 GFLOP of
    expert FFNs), token gather per expert.
  - Device (8 NeuronCores, SPMD): each core runs 9 "units": 8 routed-expert
    slots (64 slots globally = 63 experts + 1 overflow slot) and 1
    shared-expert slot over a 1/8 token slice.
    Routed units run in fp8e4m3 with DoubleRow matmuls (2 fp8 weights per PE
    cell, K=256 per instruction): h = gelu((XeT^T @ (256*W1))/256 + b1);
    y*256 = h @ (256*W2).  Weights are pre-scaled by 256 (a power of two, so
    exact) to keep them in e4m3's normal range; the 1/256 is folded into the
    GELU's input scale on layer 1 and into the host-side gate scaling on
    layer 2.  The shared-expert unit (gate 1.0, so it dominates the error
    budget) runs in plain fp16 and is scheduled FIRST: it is DMA-light and
    compute-heavy, which lets the fp8 units' weight streams run ahead.
    All of a unit's remaining DMAs plus the next unit's input DMAs are
    issued at the top of each unit (software pipeline) so the PE never
    stalls at unit boundaries.  A short burst of dummy matmuls at t=0 warms
    the PE HAM clock gate (1.2 -> 2.4 GHz) before the first real matmul.
  - Host: scatter-add gated expert outputs (+ gate*b2), add shared out,
    bias and residual.

Experts are assigned to slots by descending load rank with static per-unit
token capacities (CAPS); both matmul layers' free dim is the capacity, so
PE cost tracks actual expert load.  Overload spills into the spare 64th
slot and, beyond that, to an exact host-side FFN for the few excess
tokens.  Gating and b2 are applied on the host during the scatter.
"""

import numpy as np

B, S, HID = 2, 2048, 1280
E = 63
I = 1280
TOP_K = 7
NCORES = 8
UNITS = 9          # 8 routed-expert slots + 1 shared-expert slot
RUNITS = 8         # routed units per core
C = 512            # token capacity per expert slot
KO = HID // 128    # 10 contraction chunks of 128
KP = KO // 2       # 5 DoubleRow contraction pairs (K=256 each)
T = B * S          # 4096
TSH = T // NCORES  # 512 shared-expert tokens per core

W1CW = 256          # w1 chunk width along I (2 lhsT column groups)
W2CW = 256          # w2 chunk width along H (2 lhsT column groups)
N_W1C = I // W1CW   # 5
N_W2C = HID // W2CW  # 5

WSCALE = 256.0      # power-of-two pre-scale for fp8 routed weights
WARM_MMS = 12       # dummy matmuls to open the PE HAM clock gate

# Per-unit-index token capacities (unit 8 = shared). Experts are assigned
# to slots by load rank (rank r -> core r%8, unit r//8), so unit j only
# ever sees the j-th bucket of the descending load distribution; caps hug
# the bucket maxima of near-uniform routing. Uncovered overflow goes to
# the spare slot 63 and, beyond that, to an exact host fallback.
CAPS = [512, 492, 476, 464, 452, 440, 432, 424, C]

_cache = {}


def _build_nc():
    import concourse.mybir as mybir
    import concourse.tile as tile
    from concourse import bacc

    f32 = mybir.dt.float32
    f16 = mybir.dt.float16
    f8 = mybir.dt.float8e4
    GELU = mybir.ActivationFunctionType.Gelu
    DR = mybir.MatmulPerfMode.DoubleRow

    nc = bacc.Bacc(None, target_bir_lowering=False)

    # routed (fp8) inputs
    xg_d = nc.dram_tensor("xg", [RUNITS, 128, KO, C], f8, kind="ExternalInput")
    w1_d = nc.dram_tensor("w1", [RUNITS, N_W1C, 128, KO, W1CW], f8,
                          kind="ExternalInput")
    w2_d = nc.dram_tensor("w2", [RUNITS, N_W2C, 128, KO, W2CW], f8,
                          kind="ExternalInput")
    # shared (fp16) inputs
    xs_d = nc.dram_tensor("xs", [128, KO, TSH], f16, kind="ExternalInput")
    w1s_d = nc.dram_tensor("w1s", [N_W1C, 128, KO, W1CW], f16,
                           kind="ExternalInput")
    w2s_d = nc.dram_tensor("w2s", [N_W2C, 128, KO, W2CW], f16,
                           kind="ExternalInput")
    b1_d = nc.dram_tensor("b1", [UNITS, 128, KO], f32, kind="ExternalInput")
    # transposed output: out[u, p, hk, c] = yscaled[token c, h = hk*128+p]
    # routed units hold 256*y (host folds 1/256 into gates); shared holds y.
    out_d = nc.dram_tensor("out", [UNITS, 128, KO, C], f16, kind="ExternalOutput")

    order = [8] + list(range(RUNITS))   # shared first

    def w1_src(u, ic):
        return w1s_d[ic] if u == 8 else w1_d[u, ic]

    def w2_src(u, ic):
        return w2s_d[ic] if u == 8 else w2_d[u, ic]

    with tile.TileContext(nc) as tc:
        with tc.tile_pool(name="xg_p", bufs=3) as xg_p, \
             tc.tile_pool(name="h1_p", bufs=2) as h1_p, \
             tc.tile_pool(name="w1_p", bufs=8) as w1_p, \
             tc.tile_pool(name="w2_p", bufs=6) as w2_p, \
             tc.tile_pool(name="out_p", bufs=2) as out_p, \
             tc.tile_pool(name="sm_p", bufs=3) as sm_p, \
             tc.tile_pool(name="wm_p", bufs=1) as wm_p, \
             tc.tile_pool(name="ps1_p", bufs=3, space="PSUM") as ps1_p, \
             tc.tile_pool(name="ps2_p", bufs=4, space="PSUM") as ps2_p, \
             tc.tile_pool(name="psw_p", bufs=1, space="PSUM") as psw_p:

            # ---- PE warm-up: open the HAM clock gate while input DMAs run
            wz = wm_p.tile([128, C], f16, tag="wz")
            nc.any.memset(wz[:], 0)
            psw = psw_p.tile([128, C], f32, tag="psw")
            for _ in range(WARM_MMS):
                nc.tensor.matmul(psw[:], wz[:, :128], wz[:], start=True,
                                 stop=True)

            # per-unit tiles created by the prefetch pipeline
            st = {u: {} for u in order}

            def issue_front(u, first=False):
                """xu halves + b1 + w1 chunks 0-2 for unit u."""
                mdt = f16 if u == 8 else f8
                CAP = CAPS[u]
                d = st[u]
                d["xu"] = xu = xg_p.tile([128, KO, C], mdt, tag="xu", name="xu")
                if first:
                    # finer split so the very first matmuls start sooner
                    pieces = [(0, 2), (2, KO // 2), (KO // 2, KO)]
                else:
                    pieces = [(0, KO // 2), (KO // 2, KO)]
                for a, b in pieces:
                    if u == 8:
                        nc.sync.dma_start(xu[:, a:b, :CAP], xs_d[:, a:b, :CAP])
                    else:
                        nc.sync.dma_start(xu[:, a:b, :CAP], xg_d[u, :, a:b, :CAP])
                d["b1"] = b1u = sm_p.tile([128, KO], f32, tag="b1u", name="b1u")
                nc.sync.dma_start(b1u[:], b1_d[u])
                d["w1"] = {}
                for ic in range(3):
                    w1c = w1_p.tile([128, KO, W1CW], mdt, tag="w1c", name="w1c")
                    if first and ic == 0:
                        nc.sync.dma_start(w1c[:, :2], w1_src(u, ic)[:, :2])
                        nc.sync.dma_start(w1c[:, 2:], w1_src(u, ic)[:, 2:])
                    else:
                        nc.sync.dma_start(w1c[:], w1_src(u, ic))
                    d["w1"][ic] = w1c

            issue_front(order[0], first=True)

            for pi, u in enumerate(order):
                CAP = CAPS[u]
                shared = (u == 8)
                mdt = f16 if shared else f8
                d = st[u]
                nxt = order[pi + 1] if pi + 1 < len(order) else None

                # ---- top-of-unit DMA issue (software pipeline) ----
                # current unit's remaining w1 chunks (needed mid-mm1)
                for ic in (3, 4):
                    w1c = w1_p.tile([128, KO, W1CW], mdt, tag="w1c", name="w1c")
                    nc.sync.dma_start(w1c[:], w1_src(u, ic))
                    d["w1"][ic] = w1c
                # current unit's first w2 chunks (needed at mm2 start)
                d["w2"] = {}
                for ic in range(3):
                    w2c = w2_p.tile([128, KO, W2CW], mdt, tag="w2c", name="w2c")
                    nc.sync.dma_start(w2c[:], w2_src(u, ic))
                    d["w2"][ic] = w2c
                # next unit's inputs (needed at next unit start)
                if nxt is not None:
                    issue_front(nxt)
                # current unit's last w2 chunks
                for ic in (3, 4):
                    w2c = w2_p.tile([128, KO, W2CW], mdt, tag="w2c", name="w2c")
                    nc.sync.dma_start(w2c[:], w2_src(u, ic))
                    d["w2"][ic] = w2c

                xu = d["xu"]
                b1u = d["b1"]
                h1 = h1_p.tile([128, KO, C], mdt, tag="h1")

                # ---- mm1: h1[i, c] = gelu(sum_h W1[h,i] * X^T[h,c] + b1[i])
                for ic in range(N_W1C):
                    w1c = d["w1"][ic]
                    for s in range(W1CW // 128):
                        i_out = ic * (W1CW // 128) + s
                        ps = ps1_p.tile([128, C], f32, tag="ps1")
                        if shared:
                            for ko in range(KO):
                                nc.tensor.matmul(
                                    ps[:, :CAP],
                                    w1c[:, ko, s * 128:(s + 1) * 128],
                                    xu[:, ko, :CAP],
                                    start=(ko == 0),
                                    stop=(ko == KO - 1),
                                )
                        else:
                            for j in range(KP):
                                nc.tensor.matmul(
                                    ps[:, :CAP],
                                    w1c[:, 2 * j:2 * j + 2, s * 128:(s + 1) * 128],
                                    xu[:, 2 * j:2 * j + 2, :CAP],
                                    start=(j == 0),
                                    stop=(j == KP - 1),
                                    perf_mode=DR,
                                )
                        nc.scalar.activation(
                            h1[:, i_out, :CAP], ps[:, :CAP], GELU,
                            bias=b1u[:, i_out:i_out + 1],
                            scale=1.0 if shared else 1.0 / WSCALE)

                # ---- mm2 (transposed): yT[h, c] = sum_i W2[i, h] * h1[i, c]
                # gating and b2 are applied on the host during scatter.
                oy = out_p.tile([128, KO, C], f16, tag="oy")
                for hcc in range(N_W2C):
                    w2c = d["w2"][hcc]
                    for s2 in range(W2CW // 128):
                        hk = hcc * (W2CW // 128) + s2
                        ps2 = ps2_p.tile([128, C], f32, tag="ps2")
                        if shared:
                            for ko in range(KO):
                                nc.tensor.matmul(
                                    ps2[:, :CAP],
                                    w2c[:, ko, s2 * 128:(s2 + 1) * 128],
                                    h1[:, ko, :CAP],
                                    start=(ko == 0),
                                    stop=(ko == KO - 1),
                                )
                        else:
                            for j in range(KP):
                                nc.tensor.matmul(
                                    ps2[:, :CAP],
                                    w2c[:, 2 * j:2 * j + 2, s2 * 128:(s2 + 1) * 128],
                                    h1[:, 2 * j:2 * j + 2, :CAP],
                                    start=(j == 0),
                                    stop=(j == KP - 1),
                                    perf_mode=DR,
                                )
                        nc.vector.tensor_copy(oy[:, hk, :CAP], ps2[:, :CAP])
                        # drain finished output rows early so the final DMA
                        # (and the kernel tail) stays small
                        if hk % 2 == 1:
                            nc.sync.dma_start(
                                out_d[u, :, hk - 1:hk + 1, :CAP],
                                oy[:, hk - 1:hk + 1, :CAP])

    nc.compile()
    return nc


def _get_nc():
    if "nc" not in _cache:
        _cache["nc"] = _build_nc()
    return _cache["nc"]


def _f8():
    import ml_dtypes
    return np.dtype(ml_dtypes.float8_e4m3)


def _gelu_np(v):
    from scipy.special import erf
    v = v.astype(np.float32)
    return (0.5 * v * (1.0 + erf(v / np.sqrt(2.0)))).astype(np.float32)


def _tile_w1(w):
    # [H, I] -> [N_W1C, 128, KO, W1CW] with w1t[ic, p, ko, j] = w[ko*128+p, ic*W1CW+j]
    return w.reshape(KO, 128, N_W1C, W1CW).transpose(2, 1, 0, 3)


def _tile_w2(w):
    # [I, H] -> [N_W2C, 128, KO, W2CW]
    return w.reshape(KO, 128, N_W2C, W2CW).transpose(2, 1, 0, 3)


def _ensure_axon_hooks_stub():
    """bass_utils' axon trace path imports antenv.axon_hooks, which this
    image lacks; provide a no-op stub so a BASS_TRACE-enabled environment
    degrades gracefully instead of crashing."""
    import sys
    import types
    try:
        import antenv.axon_hooks  # noqa: F401
        return
    except ImportError:
        pass
    try:
        import antenv
    except ImportError:
        return
    mod = types.ModuleType("antenv.axon_hooks")
    holder = [None]
    mod.set_axon_ntff_profile_hook = lambda h: holder.__setitem__(0, h)
    mod.get_axon_ntff_profile_hook = lambda: holder[0]
    sys.modules["antenv.axon_hooks"] = mod
    antenv.axon_hooks = mod


def kernel(x, w1_shared, b1_shared, w2_shared, b2_shared,
           router_w, router_b, w1, b1, w2, b2):
    _ensure_axon_hooks_stub()
    from concourse.bass_utils import run_bass_kernel_spmd

    f8 = _f8()

    x = np.asarray(x, np.float32)
    w1 = np.asarray(w1, np.float32)
    b1 = np.asarray(b1, np.float32)
    w2 = np.asarray(w2, np.float32)
    b2 = np.asarray(b2, np.float32)
    w1_shared = np.asarray(w1_shared, np.float32)
    b1_shared = np.asarray(b1_shared, np.float32)
    w2_shared = np.asarray(w2_shared, np.float32)
    b2_shared = np.asarray(b2_shared, np.float32)
    router_w = np.asarray(router_w, np.float32)
    router_b = np.asarray(router_b, np.float32)

    xf = x.reshape(T, HID)

    # ---------------- host routing ----------------
    logits = xf @ router_w + router_b
    m = logits.max(-1, keepdims=True)
    ex = np.exp(logits - m, dtype=np.float32)
    affin = ex / ex.sum(-1, keepdims=True, dtype=np.float32)
    order = np.argsort(-affin, axis=-1, kind="stable")[:, :TOP_K]   # [T, K]
    vals = np.take_along_axis(affin, order, axis=-1)                # [T, K]

    # group (token, gate) pairs by expert
    flat_e = order.ravel()
    flat_t = np.repeat(np.arange(T), TOP_K)
    flat_g = vals.ravel()
    sort = np.argsort(flat_e, kind="stable")
    se, st, sg = flat_e[sort], flat_t[sort], flat_g[sort]
    starts = np.searchsorted(se, np.arange(E + 1))
    tok_by_e = [st[starts[e]:starts[e + 1]] for e in range(E)]
    gate_by_e = [sg[starts[e]:starts[e + 1]] for e in range(E)]

    # slot table: 64 expert slots; slot s = core*8 + unit.  Experts are
    # assigned by descending load rank: rank r -> core r%8, unit r//8, so
    # every core gets one expert from each load bucket and unit j's static
    # capacity CAPS[j] covers its bucket maximum.
    NSLOT = NCORES * 8
    slot_expert = [-1] * NSLOT
    slot_tok = [np.empty(0, np.int64)] * NSLOT
    slot_gate = [np.empty(0, np.float32)] * NSLOT
    ranked = sorted(range(E), key=lambda e: -len(tok_by_e[e]))
    overflow = []   # (expert, tokens, gates) beyond the primary slot cap
    for r, e in enumerate(ranked):
        s = (r % NCORES) * 8 + (r // NCORES)
        cap = CAPS[r // NCORES]
        slot_expert[s] = e
        slot_tok[s] = tok_by_e[e][:cap]
        slot_gate[s] = gate_by_e[e][:cap]
        if len(tok_by_e[e]) > cap:
            overflow.append((e, tok_by_e[e][cap:], gate_by_e[e][cap:]))
    # worst overflow spills into the spare slot 63 (unit 7, cap CAPS[7]);
    # anything further goes to an exact host fallback (rare).
    host_fallback = []
    if overflow:
        overflow.sort(key=lambda t: -len(t[1]))
        e0, t0, g0 = overflow[0]
        cap63 = CAPS[7]
        slot_expert[63] = e0
        slot_tok[63] = t0[:cap63]
        slot_gate[63] = g0[:cap63]
        if len(t0) > cap63:
            host_fallback.append((e0, t0[cap63:], g0[cap63:]))
        for e, t, g in overflow[1:]:
            host_fallback.append((e, t, g))

    # ---------------- build per-core device inputs ----------------
    # x transposed + partition-tiled: xT_t[ko, p, t] = x[t, ko*128+p]
    xT = np.ascontiguousarray(xf.T)
    xT_t8 = xT.astype(f8).reshape(KO, 128, T)
    xT_t16 = xT.astype(np.float16).reshape(KO, 128, T)

    w1t_sh = _tile_w1(w1_shared[0]).astype(np.float16)
    w2t_sh = _tile_w2(w2_shared[0]).astype(np.float16)
    b1t_sh = b1_shared[0].reshape(KO, 128).T

    in_maps = []
    for c in range(NCORES):
        xg = np.zeros((RUNITS, 128, KO, C), f8)
        w1u = np.zeros((RUNITS, N_W1C, 128, KO, W1CW), f8)
        b1u = np.zeros((UNITS, 128, KO), np.float32)
        w2u = np.zeros((RUNITS, N_W2C, 128, KO, W2CW), f8)
        for u in range(RUNITS):
            s = c * 8 + u
            e = slot_expert[s]
            if e < 0 or len(slot_tok[s]) == 0:
                continue
            n = len(slot_tok[s])
            idx = np.zeros(C, np.int64)
            idx[:n] = slot_tok[s]
            xg[u] = xT_t8[:, :, idx].swapaxes(0, 1)
            w1u[u] = _tile_w1(w1[e] * WSCALE).astype(f8)
            b1u[u] = b1[e].reshape(KO, 128).T
            w2u[u] = _tile_w2(w2[e] * WSCALE).astype(f8)
        # shared-expert unit
        xs = xT_t16[:, :, c * TSH:(c + 1) * TSH].swapaxes(0, 1)
        b1u[8] = b1t_sh
        in_maps.append({"xg": xg, "w1": w1u, "b1": b1u, "w2": w2u,
                        "xs": np.ascontiguousarray(xs),
                        "w1s": w1t_sh, "w2s": w2t_sh})

    # ---------------- run on 8 cores ----------------
    nc = _get_nc()
    res = run_bass_kernel_spmd(nc, in_maps, core_ids=list(range(NCORES)))
    outs = [r["out"] for r in res.results]   # [UNITS, 128, KO, C] each

    # ---------------- host unshard / scatter ----------------
    # device output is transposed: outs[c][u][p, hk, c'] = ysc[c', hk*128+p]
    def untile_y(o, n):
        return o.transpose(1, 0, 2).reshape(HID, C)[:, :n].T.astype(np.float32)

    acc = np.zeros((T, HID), np.float32)     # shared + routed
    # shared expert (unit 8 on each core), gate 1, + b2_shared
    for c in range(NCORES):
        ys = untile_y(outs[c][8], TSH)
        acc[c * TSH:(c + 1) * TSH] = ys + b2_shared[0]
    # routed experts: gate * (y + b2), scattered by token; device holds
    # 256*y so fold the 1/256 into the gate.
    inv = np.float32(1.0 / WSCALE)
    for s in range(NCORES * 8):
        e = slot_expert[s]
        n = len(slot_tok[s])
        if e < 0 or n == 0:
            continue
        ye = untile_y(outs[s // 8][s % 8], n)
        # token indices are unique within one slot, so fancy += is safe
        acc[slot_tok[s]] += (slot_gate[s] * inv)[:, None] * ye \
            + slot_gate[s][:, None] * b2[e][None, :]
    # exact host fallback for overflow beyond device capacity
    for e, toks, gs in host_fallback:
        h = _gelu_np(xf[toks] @ w1[e] + b1[e])
        acc[toks] += gs[:, None] * (h @ w2[e] + b2[e])

    return (acc + xf).reshape(B, S, HID).astype(np.float32)
